# revision 1
# baseline (speedup 1.0000x reference)
"""HSIC loss kernel for Trainium2, SPMD over 8 NeuronCores.

Math (reference): K = exp(-d2(x)), L = exp(-d2(y)),
  hsic = (sum(L*K) - 2*dot(rK,rL)/m + sum(K)*sum(L)/m^2) / (m-1)^2
where rK_i = sum_j K_ij (row sums; K, L symmetric).

Sharding: rows of the Gram matrices are split into 8 strips of 1024.
Each core computes its [1024, 8192] strips of K and L fully fused
(never materialized in DRAM):
  PSUM = x_strip @ x_full^T  (bf16 matmul, D=128 contraction)
         + rank-2 correction folding in -sq_j/2 (bf16 hi/lo split)
  K    = ACT exp(2*PSUM - sq_i)  (per-partition bias, scale=2)
with the exact diagonal (K_ii = exp(0) = 1) excluded in-kernel (a
-30000 "staircase" added on the diagonal before exp drives those
entries to exactly 0) and re-added analytically on the host - this is
exact math, not an approximation, and it removes any precision demand
on the d2 diagonal.

Per-core outputs: row sums of K and L (diagonal excluded) and the
running sum of K*L (diagonal excluded). Host combines in float64.

Column layout trick for SPMD uniformity: each core's moving operand
(x_full^T) is rotated so its own strip lands at columns 0..1023; the
diagonal is then at a static position for every core. Row sums are
column-order invariant.
"""

import numpy as np
import ml_dtypes

BF16 = ml_dtypes.bfloat16

M = 8192
D = 128
NDEV = 8
STRIP = M // NDEV          # 1024 rows per core
NCHUNK = STRIP // 128      # 8 partition chunks per strip
SUPER = 2048               # ACT/PSUM super-tile width (4 PSUM banks)
NSUP = M // SUPER          # 4 j-supers
TS = 512                   # matmul free-dim tile (one PSUM bank)
BIG = -30000.0             # diagonal staircase; exp(2*BIG) == 0 in fp32

_cache = {}

# Feature flags for HW bisection.
# slk_mode: "stt" = fused scalar_tensor_tensor w/ accum, "split" =
# tensor_mul + tensor_reduce, "ttr" = tensor_tensor_reduce (fails on
# this HW/runtime combo).
OPTS = {"slk_mode": "stt", "use_accum": True, "use_stair": True,
        "repeat": 1}


def _build_program():
    import concourse.bacc as bacc
    import concourse.mybir as mybir
    from concourse import tile

    f32 = mybir.dt.float32
    bf16 = mybir.dt.bfloat16
    Exp = mybir.ActivationFunctionType.Exp
    mult = mybir.AluOpType.mult
    add = mybir.AluOpType.add

    nc = bacc.Bacc("TRN2", target_bir_lowering=False, debug=False,
                   num_devices=NDEV)

    # DRAM inputs (per-core values differ, same shapes: SPMD)
    xTm_d = nc.dram_tensor("xTm", [D, M], bf16, kind="ExternalInput")
    yTm_d = nc.dram_tensor("yTm", [D, M], bf16, kind="ExternalInput")
    xTs_d = nc.dram_tensor("xTs", [D, STRIP], bf16, kind="ExternalInput")
    yTs_d = nc.dram_tensor("yTs", [D, STRIP], bf16, kind="ExternalInput")
    r2x_d = nc.dram_tensor("r2x", [2, M], bf16, kind="ExternalInput")
    r2y_d = nc.dram_tensor("r2y", [2, M], bf16, kind="ExternalInput")
    ones2_d = nc.dram_tensor("ones2", [2, D], bf16, kind="ExternalInput")
    nsqx_d = nc.dram_tensor("nsqx", [128, NCHUNK], f32, kind="ExternalInput")
    nsqy_d = nc.dram_tensor("nsqy", [128, NCHUNK], f32, kind="ExternalInput")
    stair_d = nc.dram_tensor("stair", [128, 128], f32, kind="ExternalInput")

    orK_d = nc.dram_tensor("orK", [128, NCHUNK], f32, kind="ExternalOutput")
    orL_d = nc.dram_tensor("orL", [128, NCHUNK], f32, kind="ExternalOutput")
    oS_d = nc.dram_tensor("oS", [128, 1], f32, kind="ExternalOutput")

    NSLOT = NCHUNK * NSUP  # 32 accumulation slots

    with tile.TileContext(nc) as tc:
        with (
            tc.tile_pool(name="const", bufs=1) as cpool,
            tc.tile_pool(name="psum", bufs=2, space="PSUM") as pspool,
            tc.tile_pool(name="kl", bufs=2) as klpool,
            tc.tile_pool(name="scr", bufs=2) as scrpool,
        ):
            xTm = cpool.tile([D, M], bf16, tag="xTm")
            yTm = cpool.tile([D, M], bf16, tag="yTm")
            xTs = cpool.tile([D, STRIP], bf16, tag="xTs")
            yTs = cpool.tile([D, STRIP], bf16, tag="yTs")
            r2x = cpool.tile([2, M], bf16, tag="r2x")
            r2y = cpool.tile([2, M], bf16, tag="r2y")
            ones2 = cpool.tile([2, D], bf16, tag="ones2")
            nsqx = cpool.tile([128, NCHUNK], f32, tag="nsqx")
            nsqy = cpool.tile([128, NCHUNK], f32, tag="nsqy")
            stair = cpool.tile([128, 128], f32, tag="stair")
            accK = cpool.tile([128, NSLOT], f32, tag="accK")
            accL = cpool.tile([128, NSLOT], f32, tag="accL")
            accS = cpool.tile([128, NSLOT], f32, tag="accS")
            chainS = cpool.tile([128, NSLOT + 1], f32, tag="chainS")
            oS_sb = cpool.tile([128, 1], f32, tag="oS")
            orK_sb = cpool.tile([128, NCHUNK], f32, tag="orK")
            orL_sb = cpool.tile([128, NCHUNK], f32, tag="orL")
            t1 = cpool.tile([128, NCHUNK], f32, tag="t1")
            t2 = cpool.tile([128, NCHUNK], f32, tag="t2")

            # Input DMAs (moving operands split per super for early start)
            for s in range(NSUP):
                sl = slice(s * SUPER, (s + 1) * SUPER)
                nc.gpsimd.dma_start(out=xTm[:, sl], in_=xTm_d[:, sl])
                nc.gpsimd.dma_start(out=yTm[:, sl], in_=yTm_d[:, sl])
            nc.gpsimd.dma_start(out=xTs[:, :], in_=xTs_d[:, :])
            nc.gpsimd.dma_start(out=yTs[:, :], in_=yTs_d[:, :])
            nc.gpsimd.dma_start(out=r2x[:, :], in_=r2x_d[:, :])
            nc.gpsimd.dma_start(out=r2y[:, :], in_=r2y_d[:, :])
            nc.gpsimd.dma_start(out=ones2[:, :], in_=ones2_d[:, :])
            nc.gpsimd.dma_start(out=nsqx[:, :], in_=nsqx_d[:, :])
            nc.gpsimd.dma_start(out=nsqy[:, :], in_=nsqy_d[:, :])
            nc.gpsimd.dma_start(out=stair[:, :], in_=stair_d[:, :])

            nc.vector.memset(chainS[:, 0:1], 0.0)

            # body emitted OPTS["repeat"] times (>1 only for HW timing:
            # outputs are identical per repeat, slope gives body time)
            for c in range(NCHUNK * OPTS["repeat"]):
                c = c % NCHUNK
                cs = slice(c * 128, (c + 1) * 128)
                for s in range(NSUP):
                    slot = s * NCHUNK + c       # acc layout: s-major
                    link = c * NSUP + s         # chain order: loop order
                    psK = pspool.tile([128, SUPER], f32, tag="ps")
                    psL = pspool.tile([128, SUPER], f32, tag="ps")
                    for t in range(NSUP):
                        jsl = slice(s * SUPER + t * TS, s * SUPER + (t + 1) * TS)
                        tsl = slice(t * TS, (t + 1) * TS)
                        nc.tensor.matmul(psK[:, tsl], lhsT=xTs[:, cs],
                                         rhs=xTm[:, jsl], start=True, stop=False)
                    for t in range(NSUP):
                        jsl = slice(s * SUPER + t * TS, s * SUPER + (t + 1) * TS)
                        tsl = slice(t * TS, (t + 1) * TS)
                        nc.tensor.matmul(psK[:, tsl], lhsT=ones2[:, :],
                                         rhs=r2x[:, jsl], start=False, stop=True)
                    if s == 0 and OPTS["use_stair"]:
                        nc.vector.tensor_add(psK[:, cs], psK[:, cs], stair[:, :])
                    K_sb = klpool.tile([128, SUPER], bf16, tag="K")
                    if OPTS["use_accum"]:
                        nc.scalar.activation(K_sb[:, :], psK[:, :], Exp,
                                             bias=nsqx[:, c:c + 1], scale=2.0,
                                             accum_out=accK[:, slot:slot + 1])
                    else:
                        nc.scalar.activation(K_sb[:, :], psK[:, :], Exp,
                                             bias=nsqx[:, c:c + 1], scale=2.0)
                        nc.vector.tensor_reduce(
                            accK[:, slot:slot + 1], K_sb[:, :],
                            axis=mybir.AxisListType.X, op=add)

                    for t in range(NSUP):
                        jsl = slice(s * SUPER + t * TS, s * SUPER + (t + 1) * TS)
                        tsl = slice(t * TS, (t + 1) * TS)
                        nc.tensor.matmul(psL[:, tsl], lhsT=yTs[:, cs],
                                         rhs=yTm[:, jsl], start=True, stop=False)
                    for t in range(NSUP):
                        jsl = slice(s * SUPER + t * TS, s * SUPER + (t + 1) * TS)
                        tsl = slice(t * TS, (t + 1) * TS)
                        nc.tensor.matmul(psL[:, tsl], lhsT=ones2[:, :],
                                         rhs=r2y[:, jsl], start=False, stop=True)
                    if s == 0 and OPTS["use_stair"]:
                        nc.vector.tensor_add(psL[:, cs], psL[:, cs], stair[:, :])
                    L_sb = klpool.tile([128, SUPER], bf16, tag="L")
                    if OPTS["use_accum"]:
                        nc.scalar.activation(L_sb[:, :], psL[:, :], Exp,
                                             bias=nsqy[:, c:c + 1], scale=2.0,
                                             accum_out=accL[:, slot:slot + 1])
                    else:
                        nc.scalar.activation(L_sb[:, :], psL[:, :], Exp,
                                             bias=nsqy[:, c:c + 1], scale=2.0)
                        nc.vector.tensor_reduce(
                            accL[:, slot:slot + 1], L_sb[:, :],
                            axis=mybir.AxisListType.X, op=add)

                    scr = scrpool.tile([128, SUPER], bf16, tag="scr")
                    if OPTS["slk_mode"] == "ttr":
                        nc.vector.tensor_tensor_reduce(
                            out=scr[:, :], in0=K_sb[:, :], in1=L_sb[:, :],
                            scale=1.0, scalar=chainS[:, link:link + 1],
                            op0=mult, op1=add,
                            accum_out=chainS[:, link + 1:link + 2])
                    elif OPTS["slk_mode"] == "stt":
                        nc.vector.scalar_tensor_tensor(
                            out=scr[:, :], in0=K_sb[:, :], scalar=1.0,
                            in1=L_sb[:, :], op0=mult, op1=mult,
                            accum_out=accS[:, slot:slot + 1])
                    else:
                        nc.vector.tensor_mul(scr[:, :], K_sb[:, :], L_sb[:, :])
                        nc.vector.tensor_reduce(
                            accS[:, slot:slot + 1], scr[:, :],
                            axis=mybir.AxisListType.X, op=add)

            # orK[:, c] = sum_s accK[:, s*8 + c]  (pairwise adds on slices)
            nc.vector.tensor_add(t1[:, :], accK[:, 0:8], accK[:, 8:16])
            nc.vector.tensor_add(t2[:, :], accK[:, 16:24], accK[:, 24:32])
            nc.vector.tensor_add(orK_sb[:, :], t1[:, :], t2[:, :])
            nc.gpsimd.dma_start(out=orK_d[:, :], in_=orK_sb[:, :])

            nc.vector.tensor_add(t1[:, :], accL[:, 0:8], accL[:, 8:16])
            nc.vector.tensor_add(t2[:, :], accL[:, 16:24], accL[:, 24:32])
            nc.vector.tensor_add(orL_sb[:, :], t1[:, :], t2[:, :])
            nc.gpsimd.dma_start(out=orL_d[:, :], in_=orL_sb[:, :])

            if OPTS["slk_mode"] == "ttr":
                nc.gpsimd.dma_start(out=oS_d[:, :],
                                    in_=chainS[:, NSLOT:NSLOT + 1])
            else:
                nc.vector.tensor_add(t1[:, :], accS[:, 0:8], accS[:, 8:16])
                nc.vector.tensor_add(t2[:, :], accS[:, 16:24], accS[:, 24:32])
                nc.vector.tensor_add(t1[:, :], t1[:, :], t2[:, :])
                nc.vector.tensor_reduce(oS_sb[:, :], t1[:, :],
                                        axis=mybir.AxisListType.X, op=add)
                nc.gpsimd.dma_start(out=oS_d[:, :], in_=oS_sb[:, :])

    nc.compile()
    return nc


def _get_program():
    key = tuple(sorted(OPTS.items()))
    if key not in _cache:
        _cache[key] = _build_program()
    return _cache[key]


def _prep_core_inputs(xb, yb, sqx, sqy, dev):
    """Build the per-core input dict. xb/yb: bf16 [M, D]; sqx/sqy: f64 [M]."""
    ins = {}
    for name, ab, sq in (("x", xb, sqx), ("y", yb, sqy)):
        rot = np.roll(ab, -dev * STRIP, axis=0)          # [M, D]
        ins[f"{name}Tm"] = np.ascontiguousarray(rot.T)   # [D, M] bf16
        ins[f"{name}Ts"] = np.ascontiguousarray(
            ab[dev * STRIP:(dev + 1) * STRIP].T)         # [D, STRIP] bf16
        v = -np.roll(sq, -dev * STRIP) / 2.0             # f64 [M]
        hi = v.astype(BF16)
        lo = (v - hi.astype(np.float64)).astype(BF16)
        ins[f"r2{name}"] = np.ascontiguousarray(
            np.stack([hi, lo], axis=0))                  # [2, M] bf16
        nsq = -sq[dev * STRIP:(dev + 1) * STRIP].astype(np.float32)
        ins[f"nsq{name}"] = np.ascontiguousarray(
            nsq.reshape(NCHUNK, 128).T)                  # [128, NCHUNK] f32
    ins["ones2"] = np.ones((2, D), dtype=BF16)
    ins["stair"] = (np.eye(128, dtype=np.float32) * np.float32(BIG))
    return ins


def prepare_in_maps(x, y):
    x = np.asarray(x, dtype=np.float32)
    y = np.asarray(y, dtype=np.float32)
    xb = x.astype(BF16)
    yb = y.astype(BF16)
    sqx = (xb.astype(np.float64) ** 2).sum(axis=1)       # [M] f64
    sqy = (yb.astype(np.float64) ** 2).sum(axis=1)
    return [_prep_core_inputs(xb, yb, sqx, sqy, dev) for dev in range(NDEV)]


def combine(results):
    """Host-side unshard + closed-form diagonal. float64 combine."""
    rK = np.ones(M, dtype=np.float64)
    rL = np.ones(M, dtype=np.float64)
    S_lk = np.float64(M)
    for dev in range(NDEV):
        r = results[dev]
        sl = slice(dev * STRIP, (dev + 1) * STRIP)
        rK[sl] += np.asarray(r["orK"], dtype=np.float64).T.reshape(STRIP)
        rL[sl] += np.asarray(r["orL"], dtype=np.float64).T.reshape(STRIP)
        S_lk += np.asarray(r["oS"], dtype=np.float64).sum()
    S_K = rK.sum()
    S_L = rL.sum()
    dotRR = (rK * rL).sum()
    hsic = (S_lk - 2.0 * dotRR / M + S_K * S_L / (float(M) ** 2)) \
        / float((M - 1) ** 2)
    return np.float32(hsic)


def _get_runner():
    """Build (once) a cached jitted SPMD runner over the 8 cores.

    Mirrors concourse.bass2jax.run_bass_via_pjrt but caches the jitted
    callable so repeated kernel() calls skip retrace/recompile.
    """
    rkey = ("runner",) + tuple(sorted(OPTS.items()))
    if rkey in _cache:
        return _cache[rkey]
    import jax
    import numpy as _np
    from jax.sharding import Mesh, PartitionSpec
    from jax.experimental.shard_map import shard_map
    from concourse import bass2jax as b2j
    import concourse.mybir as mybir

    b2j.install_neuronx_cc_hook()
    nc = _get_program()

    partition_name = (nc.partition_id_tensor.name
                      if nc.partition_id_tensor else None)
    in_names, out_names, out_avals, zero_outs = [], [], [], []
    for alloc in nc.m.functions[0].allocations:
        if not isinstance(alloc, mybir.MemoryLocationSet):
            continue
        name = alloc.memorylocations[0].name
        if alloc.kind == "ExternalInput":
            if name != partition_name:
                in_names.append(name)
        elif alloc.kind == "ExternalOutput":
            out_names.append(name)
            np_dt = mybir.dt.np(alloc.dtype)
            out_avals.append(jax.core.ShapedArray(
                tuple(alloc.tensor_shape), np_dt))
            zero_outs.append(_np.zeros(tuple(alloc.tensor_shape), np_dt))

    n_params = len(in_names)
    n_outs = len(out_names)
    all_names = in_names + out_names
    if partition_name is not None:
        all_names = all_names + [partition_name]

    def _body(*args):
        operands = list(args)
        if partition_name is not None:
            operands.append(b2j.partition_id_tensor())
        outs = b2j._bass_exec_p.bind(
            *operands,
            out_avals=tuple(out_avals),
            in_names=tuple(all_names),
            out_names=tuple(out_names),
            lowering_input_output_aliases=(),
            sim_require_finite=True,
            sim_require_nnan=True,
            nc=nc,
        )
        return tuple(outs)

    devices = jax.devices()[:NDEV]
    mesh = Mesh(_np.asarray(devices), ("core",))
    donate = tuple(range(n_params, n_params + n_outs))
    sharded = jax.jit(
        shard_map(_body, mesh=mesh,
                  in_specs=(PartitionSpec("core"),) * (n_params + n_outs),
                  out_specs=(PartitionSpec("core"),) * n_outs,
                  check_rep=False),
        donate_argnums=donate, keep_unused=True)

    _cache[rkey] = (sharded, in_names, out_names, out_avals, zero_outs)
    return _cache[rkey]


def run_device(in_maps, repeats=1):
    """Run the SPMD program; returns per-core output dicts (last repeat)."""
    import jax
    sharded, in_names, out_names, out_avals, zero_outs = _get_runner()
    concat_in = [
        np.concatenate([np.asarray(in_maps[c][nm]) for c in range(NDEV)],
                       axis=0)
        for nm in in_names
    ]
    dev_in = [jax.device_put(a) for a in concat_in]
    out_arrs = None
    for _ in range(repeats):
        zeros = [np.zeros((NDEV * z.shape[0], *z.shape[1:]), z.dtype)
                 for z in zero_outs]
        out_arrs = sharded(*dev_in, *zeros)
    out_arrs = [np.asarray(a) for a in out_arrs]
    return [
        {nm: out_arrs[i].reshape(NDEV, *out_avals[i].shape)[c]
         for i, nm in enumerate(out_names)}
        for c in range(NDEV)
    ]


def kernel(x, y):
    in_maps = prepare_in_maps(x, y)
    results = run_device(in_maps)
    return combine(results)


def _timed_run(in_maps, iters):
    """Min wall seconds for one dispatch of the current OPTS program."""
    import jax
    import time as _time
    sharded, in_names, out_names, out_avals, zero_outs = _get_runner()
    concat_in = [
        np.concatenate([np.asarray(in_maps[c][nm]) for c in range(NDEV)],
                       axis=0)
        for nm in in_names
    ]
    dev_in = [jax.device_put(a) for a in concat_in]
    best = float("inf")
    for i in range(iters + 1):
        zeros = [np.zeros((NDEV * z.shape[0], *z.shape[1:]), z.dtype)
                 for z in zero_outs]
        t0 = _time.perf_counter()
        outs = sharded(*dev_in, *zeros)
        [np.asarray(o) for o in outs]
        dt = _time.perf_counter() - t0
        if i > 0:  # skip warm-up/compile call
            best = min(best, dt)
    return best


def time_on_hw(in_maps, r_small=1, r_big=17, iters=8):
    """Estimate per-body HW time: (wall[R=r_big] - wall[R=r_small]) /
    (r_big - r_small), where R is the in-program body repeat count."""
    saved = OPTS["repeat"]
    walls = {}
    try:
        for r in (r_small, r_big):
            OPTS["repeat"] = r
            walls[r] = _timed_run(in_maps, iters)
    finally:
        OPTS["repeat"] = saved
    per_body = (walls[r_big] - walls[r_small]) / (r_big - r_small)
    return per_body * 1e9, walls



# revision 3
# speedup vs baseline: 6.2756x; 6.2756x over previous
"""HSIC loss kernel for Trainium2, SPMD over 8 NeuronCores.

Math (reference): K = exp(-d2(x)), L = exp(-d2(y)),
  hsic = (sum(L*K) - 2*dot(rK,rL)/m + sum(K)*sum(L)/m^2) / (m-1)^2
where rK_i = sum_j K_ij (row sums; K, L symmetric).

Sharding: rows of the Gram matrices are split into 8 strips of 1024.
Each core receives ONLY its own strip of x and y (transposed, bf16)
plus tiny per-row metadata; the full x^T/y^T moving operands are
assembled on-device with an AllGather collective. This keeps the
host->device wire traffic at ~5 MB/call (vs ~39 MB if every core's
full rotated copy were shipped from the host), which dominates the
end-to-end latency on the axon-tunneled PJRT transport.

Per core, the [1024, 8192] strips of K and L are computed fully fused
(never materialized in DRAM):
  PSUM = x_strip @ x_full^T  (bf16 matmul, D=128 contraction)
         + rank-2 correction folding in -sq_j/2 (bf16 hi/lo split)
  K    = ACT exp(2*PSUM - sq_i)  (per-partition bias, scale=2)
The diagonal needs exact treatment (off-diagonal entries are ~e-30;
the diagonal K_ii = 1 carries the whole answer). Because the strips
are gathered in natural order, the diagonal block position would be
core-dependent, which a static SPMD program cannot address. Instead
the main pass INCLUDES the (slightly inexact) diagonal, and a second
tiny pass recomputes the 8 diagonal [128,128] blocks bit-identically
from the local strip (same operand values, same accumulation order),
extracts their diagonals, and subtracts them from the row sums and
the K*L sum. The true diagonal (exp(0)=1) is re-added analytically
on the host - exact math, not an approximation.

Per-core output is a single [128, 17] f32 tensor: row sums of K and
L by chunk (diag excluded) and the K*L partial sum. Host combines in
float64.
"""

import numpy as np
import ml_dtypes

BF16 = ml_dtypes.bfloat16

M = 8192
D = 128
NDEV = 8
STRIP = M // NDEV          # 1024 rows per core
NCHUNK = STRIP // 128      # 8 partition chunks per strip
SUPER = 2048               # ACT/PSUM super-tile width (4 PSUM banks)
NSUP = M // SUPER          # 4 j-supers
TS = 512                   # matmul free-dim tile (one PSUM bank)

R2W = M + STRIP            # 9216: full-M correction row + own-strip slice
NSLOT = NCHUNK * NSUP      # 32 accumulation slots

_cache = {}

OPTS = {"repeat": 1}


def _build_program():
    import concourse.bacc as bacc
    import concourse.mybir as mybir
    from concourse import tile

    f32 = mybir.dt.float32
    bf16 = mybir.dt.bfloat16
    Exp = mybir.ActivationFunctionType.Exp
    mult = mybir.AluOpType.mult
    add = mybir.AluOpType.add

    nc = bacc.Bacc("TRN2", target_bir_lowering=False, debug=False,
                   num_devices=NDEV)

    # DRAM inputs (per-core values differ, same shapes: SPMD)
    xys_d = nc.dram_tensor("xys", [128, 2 * STRIP], bf16, kind="ExternalInput")
    r2_d = nc.dram_tensor("r2", [2, 2 * R2W], bf16, kind="ExternalInput")
    nsq_d = nc.dram_tensor("nsq", [128, 2 * NCHUNK], f32, kind="ExternalInput")
    eye_d = nc.dram_tensor("eye", [128, 128], bf16, kind="ExternalInput")

    out_d = nc.dram_tensor("out", [128, 17], f32, kind="ExternalOutput")

    with tile.TileContext(nc) as tc:
        with (
            tc.tile_pool(name="dram", bufs=1, space="DRAM") as dram,
            tc.tile_pool(name="const", bufs=1) as cpool,
            tc.tile_pool(name="psum", bufs=2, space="PSUM") as pspool,
            tc.tile_pool(name="kl", bufs=2) as klpool,
            tc.tile_pool(name="scr", bufs=2) as scrpool,
        ):
            # --- AllGather the x/y strips into full moving operands ---
            cc_in = dram.tile([128, 2 * STRIP], bf16)
            cc_out = dram.tile([NDEV * 128, 2 * STRIP], bf16)
            nc.gpsimd.dma_start(out=cc_in[:, :], in_=xys_d[:, :])
            nc.gpsimd.collective_compute(
                "AllGather",
                mybir.AluOpType.bypass,
                replica_groups=[list(range(NDEV))],
                ins=[cc_in.opt()],
                outs=[cc_out.opt()],
            )

            xys = cpool.tile([128, 2 * STRIP], bf16, tag="xys")
            r2x = cpool.tile([2, R2W], bf16, tag="r2x")
            r2y = cpool.tile([2, R2W], bf16, tag="r2y")
            nsq = cpool.tile([128, 2 * NCHUNK], f32, tag="nsq")
            eye = cpool.tile([128, 128], bf16, tag="eye")
            ones2 = cpool.tile([2, D], bf16, tag="ones2")
            xG = cpool.tile([128, M], bf16, tag="xG")
            yG = cpool.tile([128, M], bf16, tag="yG")
            accK = cpool.tile([128, NSLOT], f32, tag="accK")
            accL = cpool.tile([128, NSLOT], f32, tag="accL")
            accS = cpool.tile([128, NSLOT], f32, tag="accS")
            diagK = cpool.tile([128, NCHUNK], f32, tag="diagK")
            diagL = cpool.tile([128, NCHUNK], f32, tag="diagL")
            out_sb = cpool.tile([128, 17], f32, tag="out")
            t1 = cpool.tile([128, NCHUNK], f32, tag="t1")
            t2 = cpool.tile([128, NCHUNK], f32, tag="t2")
            u1 = cpool.tile([128, NCHUNK], f32, tag="u1")
            u2 = cpool.tile([128, NCHUNK], f32, tag="u2")

            nc.gpsimd.dma_start(out=xys[:, :], in_=xys_d[:, :])
            nc.gpsimd.dma_start(out=r2x[:, :], in_=r2_d[:, 0:R2W])
            nc.gpsimd.dma_start(out=r2y[:, :], in_=r2_d[:, R2W:2 * R2W])
            nc.gpsimd.dma_start(out=nsq[:, :], in_=nsq_d[:, :])
            nc.gpsimd.dma_start(out=eye[:, :], in_=eye_d[:, :])
            nc.vector.memset(ones2[:, :], 1.0)

            # Gathered blocks -> SBUF full operands (block b at cols b*STRIP)
            for b in range(NDEV):
                rs = slice(b * 128, (b + 1) * 128)
                cs = slice(b * STRIP, (b + 1) * STRIP)
                nc.gpsimd.dma_start(out=xG[:, cs], in_=cc_out[rs, 0:STRIP])
                nc.gpsimd.dma_start(out=yG[:, cs],
                                    in_=cc_out[rs, STRIP:2 * STRIP])

            xTs = xys[:, 0:STRIP]
            yTs = xys[:, STRIP:2 * STRIP]

            # body emitted OPTS["repeat"] times (>1 only for HW timing:
            # outputs are identical per repeat, slope gives body time)
            for c in range(NCHUNK * OPTS["repeat"]):
                c = c % NCHUNK
                cs = slice(c * 128, (c + 1) * 128)
                for s in range(NSUP):
                    slot = s * NCHUNK + c       # acc layout: s-major
                    psK = pspool.tile([128, SUPER], f32, tag="ps")
                    psL = pspool.tile([128, SUPER], f32, tag="ps")
                    for t in range(NSUP):
                        jsl = slice(s * SUPER + t * TS, s * SUPER + (t + 1) * TS)
                        tsl = slice(t * TS, (t + 1) * TS)
                        nc.tensor.matmul(psK[:, tsl], lhsT=xTs[:, cs],
                                         rhs=xG[:, jsl], start=True, stop=False)
                    for t in range(NSUP):
                        jsl = slice(s * SUPER + t * TS, s * SUPER + (t + 1) * TS)
                        tsl = slice(t * TS, (t + 1) * TS)
                        nc.tensor.matmul(psK[:, tsl], lhsT=ones2[:, :],
                                         rhs=r2x[:, jsl], start=False, stop=True)
                    K_sb = klpool.tile([128, SUPER], bf16, tag="K")
                    nc.scalar.activation(K_sb[:, :], psK[:, :], Exp,
                                         bias=nsq[:, c:c + 1], scale=2.0,
                                         accum_out=accK[:, slot:slot + 1])

                    for t in range(NSUP):
                        jsl = slice(s * SUPER + t * TS, s * SUPER + (t + 1) * TS)
                        tsl = slice(t * TS, (t + 1) * TS)
                        nc.tensor.matmul(psL[:, tsl], lhsT=yTs[:, cs],
                                         rhs=yG[:, jsl], start=True, stop=False)
                    for t in range(NSUP):
                        jsl = slice(s * SUPER + t * TS, s * SUPER + (t + 1) * TS)
                        tsl = slice(t * TS, (t + 1) * TS)
                        nc.tensor.matmul(psL[:, tsl], lhsT=ones2[:, :],
                                         rhs=r2y[:, jsl], start=False, stop=True)
                    L_sb = klpool.tile([128, SUPER], bf16, tag="L")
                    nc.scalar.activation(L_sb[:, :], psL[:, :], Exp,
                                         bias=nsq[:, NCHUNK + c:NCHUNK + c + 1],
                                         scale=2.0,
                                         accum_out=accL[:, slot:slot + 1])

                    scr = scrpool.tile([128, SUPER], bf16, tag="scr")
                    nc.vector.scalar_tensor_tensor(
                        out=scr[:, :], in0=K_sb[:, :], scalar=1.0,
                        in1=L_sb[:, :], op0=mult, op1=mult,
                        accum_out=accS[:, slot:slot + 1])

            # --- pass B: recompute diagonal blocks bit-identically from the
            # local strip and extract their diagonals ---
            psDK = pspool.tile([128, SUPER], f32, tag="ps")
            psDL = pspool.tile([128, SUPER], f32, tag="ps")
            for c in range(NCHUNK):
                cs = slice(c * 128, (c + 1) * 128)
                nc.tensor.matmul(psDK[:, cs], lhsT=xTs[:, cs], rhs=xTs[:, cs],
                                 start=True, stop=False)
                nc.tensor.matmul(psDK[:, cs], lhsT=ones2[:, :],
                                 rhs=r2x[:, M + c * 128:M + (c + 1) * 128],
                                 start=False, stop=True)
                nc.tensor.matmul(psDL[:, cs], lhsT=yTs[:, cs], rhs=yTs[:, cs],
                                 start=True, stop=False)
                nc.tensor.matmul(psDL[:, cs], lhsT=ones2[:, :],
                                 rhs=r2y[:, M + c * 128:M + (c + 1) * 128],
                                 start=False, stop=True)
            KD = klpool.tile([128, SUPER], bf16, tag="K")
            LD = klpool.tile([128, SUPER], bf16, tag="L")
            for c in range(NCHUNK):
                cs = slice(c * 128, (c + 1) * 128)
                nc.scalar.activation(KD[:, cs], psDK[:, cs], Exp,
                                     bias=nsq[:, c:c + 1], scale=2.0)
                nc.scalar.activation(LD[:, cs], psDL[:, cs], Exp,
                                     bias=nsq[:, NCHUNK + c:NCHUNK + c + 1],
                                     scale=2.0)
            scrD = scrpool.tile([128, SUPER], bf16, tag="scr")
            for c in range(NCHUNK):
                cs = slice(c * 128, (c + 1) * 128)
                nc.vector.scalar_tensor_tensor(
                    out=scrD[:, cs], in0=KD[:, cs], scalar=1.0,
                    in1=eye[:, :], op0=mult, op1=mult,
                    accum_out=diagK[:, c:c + 1])
                nc.vector.scalar_tensor_tensor(
                    out=scrD[:, cs], in0=LD[:, cs], scalar=1.0,
                    in1=eye[:, :], op0=mult, op1=mult,
                    accum_out=diagL[:, c:c + 1])

            # --- final reductions: out[:, c] = sum_s acc[:, s*8+c] - diag ---
            nc.vector.tensor_add(t1[:, :], accK[:, 0:8], accK[:, 8:16])
            nc.vector.tensor_add(t2[:, :], accK[:, 16:24], accK[:, 24:32])
            nc.vector.tensor_add(t1[:, :], t1[:, :], t2[:, :])
            nc.vector.tensor_sub(out_sb[:, 0:8], t1[:, :], diagK[:, :])

            nc.vector.tensor_add(u1[:, :], accL[:, 0:8], accL[:, 8:16])
            nc.vector.tensor_add(u2[:, :], accL[:, 16:24], accL[:, 24:32])
            nc.vector.tensor_add(u1[:, :], u1[:, :], u2[:, :])
            nc.vector.tensor_sub(out_sb[:, 8:16], u1[:, :], diagL[:, :])

            nc.vector.tensor_add(t1[:, :], accS[:, 0:8], accS[:, 8:16])
            nc.vector.tensor_add(t2[:, :], accS[:, 16:24], accS[:, 24:32])
            nc.vector.tensor_add(t1[:, :], t1[:, :], t2[:, :])
            nc.vector.tensor_mul(t2[:, :], diagK[:, :], diagL[:, :])
            nc.vector.tensor_sub(t1[:, :], t1[:, :], t2[:, :])
            nc.vector.tensor_reduce(out_sb[:, 16:17], t1[:, :],
                                    axis=mybir.AxisListType.X, op=add)

            nc.gpsimd.dma_start(out=out_d[:, :], in_=out_sb[:, :])

    nc.compile()
    return nc


def _get_program():
    key = tuple(sorted(OPTS.items()))
    if key not in _cache:
        _cache[key] = _build_program()
    return _cache[key]


_EYE = None


def _eye_input():
    global _EYE
    if _EYE is None:
        _EYE = np.tile(np.eye(128, dtype=BF16), (NDEV, 1))
    return _EYE


def prepare_inputs(x, y):
    """Build the concatenated (core-major axis 0) input arrays."""
    xb = np.asarray(x, dtype=np.float32).astype(BF16)
    yb = np.asarray(y, dtype=np.float32).astype(BF16)

    XYS = np.empty((NDEV * 128, 2 * STRIP), dtype=BF16)
    R2 = np.empty((NDEV * 2, 2 * R2W), dtype=BF16)
    NSQ = np.empty((NDEV * 128, 2 * NCHUNK), dtype=np.float32)

    for off, ab in ((0, xb), (1, yb)):
        abT = np.ascontiguousarray(ab.T)                  # [128, M] bf16
        af = ab.astype(np.float32)
        sq = (af * af).sum(axis=1, dtype=np.float64)      # [M] f64
        v = -sq / 2.0
        hi = v.astype(BF16)
        lo = (v - hi.astype(np.float64)).astype(BF16)
        hilo = np.stack([hi, lo], axis=0)                 # [2, M] bf16
        nsqf = (-sq).astype(np.float32)                   # [M] f32
        for dev in range(NDEV):
            sl = slice(dev * STRIP, (dev + 1) * STRIP)
            XYS[dev * 128:(dev + 1) * 128,
                off * STRIP:(off + 1) * STRIP] = abT[:, sl]
            r2block = R2[dev * 2:(dev + 1) * 2]
            r2block[:, off * R2W:off * R2W + M] = hilo
            r2block[:, off * R2W + M:(off + 1) * R2W] = hilo[:, sl]
            NSQ[dev * 128:(dev + 1) * 128,
                off * NCHUNK:(off + 1) * NCHUNK] = \
                nsqf[sl].reshape(NCHUNK, 128).T
    return {"xys": XYS, "r2": R2, "nsq": NSQ}


def combine(out_all):
    """Host-side unshard + closed-form diagonal. float64 combine.

    out_all: [NDEV, 128, 17] f32 device results.
    """
    out_all = np.asarray(out_all, dtype=np.float64)
    rK = np.ones(M, dtype=np.float64)
    rL = np.ones(M, dtype=np.float64)
    for dev in range(NDEV):
        sl = slice(dev * STRIP, (dev + 1) * STRIP)
        rK[sl] += out_all[dev, :, 0:8].T.reshape(STRIP)
        rL[sl] += out_all[dev, :, 8:16].T.reshape(STRIP)
    S_lk = float(M) + out_all[:, :, 16].sum()
    S_K = rK.sum()
    S_L = rL.sum()
    dotRR = (rK * rL).sum()
    hsic = (S_lk - 2.0 * dotRR / M + S_K * S_L / (float(M) ** 2)) \
        / float((M - 1) ** 2)
    return np.float32(hsic)


def _get_runner():
    """Build (once) a cached jitted SPMD runner over the 8 cores.

    Constant inputs (eye) and the dummy output operand buffers are
    device-resident and reused across calls; per-call work is only the
    3 data-dependent input transfers, dispatch, and one small fetch.
    """
    rkey = ("runner",) + tuple(sorted(OPTS.items()))
    if rkey in _cache:
        return _cache[rkey]
    import jax
    import numpy as _np
    from jax.sharding import Mesh, PartitionSpec, NamedSharding
    from jax.experimental.shard_map import shard_map
    from concourse import bass2jax as b2j
    import concourse.mybir as mybir

    b2j.install_neuronx_cc_hook()
    nc = _get_program()

    partition_name = (nc.partition_id_tensor.name
                      if nc.partition_id_tensor else None)
    in_names, out_names, out_avals, zero_outs = [], [], [], []
    for alloc in nc.m.functions[0].allocations:
        if not isinstance(alloc, mybir.MemoryLocationSet):
            continue
        name = alloc.memorylocations[0].name
        if alloc.kind == "ExternalInput":
            if name != partition_name:
                in_names.append(name)
        elif alloc.kind == "ExternalOutput":
            out_names.append(name)
            np_dt = mybir.dt.np(alloc.dtype)
            out_avals.append(jax.core.ShapedArray(
                tuple(alloc.tensor_shape), np_dt))
            zero_outs.append(_np.zeros(tuple(alloc.tensor_shape), np_dt))

    n_params = len(in_names)
    all_names = list(in_names) + list(out_names)
    if partition_name is not None:
        all_names = all_names + [partition_name]

    def _body(*args):
        operands = list(args)
        if partition_name is not None:
            operands.append(b2j.partition_id_tensor())
        outs = b2j._bass_exec_p.bind(
            *operands,
            out_avals=tuple(out_avals),
            in_names=tuple(all_names),
            out_names=tuple(out_names),
            lowering_input_output_aliases=(),
            sim_require_finite=True,
            sim_require_nnan=True,
            nc=nc,
        )
        return tuple(outs)

    devices = jax.devices()[:NDEV]
    mesh = Mesh(_np.asarray(devices), ("core",))
    sharding = NamedSharding(mesh, PartitionSpec("core"))
    n_ops = n_params + len(out_names)
    sharded = jax.jit(
        shard_map(_body, mesh=mesh,
                  in_specs=(PartitionSpec("core"),) * n_ops,
                  out_specs=(PartitionSpec("core"),) * len(out_names),
                  check_rep=False),
        keep_unused=True)

    # Device-resident constants: dummy output operands + the eye input.
    zero_dev = [
        jax.device_put(_np.zeros((NDEV * z.shape[0], *z.shape[1:]), z.dtype),
                       sharding)
        for z in zero_outs
    ]
    const_dev = {"eye": jax.device_put(_eye_input(), sharding)}

    _cache[rkey] = (sharded, in_names, out_names, out_avals, zero_dev,
                    const_dev, sharding)
    return _cache[rkey]


def run_device(arrays):
    """Run the SPMD program; returns out array [NDEV, 128, 17]."""
    import jax
    (sharded, in_names, out_names, out_avals, zero_dev, const_dev,
     sharding) = _get_runner()
    dev_in = [const_dev[nm] if nm in const_dev
              else jax.device_put(arrays[nm], sharding)
              for nm in in_names]
    out_arrs = sharded(*dev_in, *zero_dev)
    out = np.asarray(out_arrs[0])
    return out.reshape(NDEV, *out_avals[0].shape)


def kernel(x, y):
    arrays = prepare_inputs(x, y)
    out = run_device(arrays)
    return combine(out)


def _timed_run(arrays, iters):
    """Min wall seconds for one dispatch of the current OPTS program."""
    import jax
    import time as _time
    (sharded, in_names, out_names, out_avals, zero_dev, const_dev,
     sharding) = _get_runner()
    dev_in = [const_dev[nm] if nm in const_dev
              else jax.device_put(arrays[nm], sharding)
              for nm in in_names]
    jax.block_until_ready(dev_in)
    best = float("inf")
    for i in range(iters + 1):
        t0 = _time.perf_counter()
        outs = sharded(*dev_in, *zero_dev)
        [np.asarray(o) for o in outs]
        dt = _time.perf_counter() - t0
        if i > 0:  # skip warm-up/compile call
            best = min(best, dt)
    return best


def time_on_hw(arrays, r_small=1, r_big=17, iters=8):
    """Estimate per-body HW time: (wall[R=r_big] - wall[R=r_small]) /
    (r_big - r_small), where R is the in-program body repeat count."""
    saved = OPTS["repeat"]
    walls = {}
    try:
        for r in (r_small, r_big):
            OPTS["repeat"] = r
            walls[r] = _timed_run(arrays, iters)
    finally:
        OPTS["repeat"] = saved
    per_body = (walls[r_big] - walls[r_small]) / (r_big - r_small)
    return per_body * 1e9, walls
